# revision 2
# baseline (speedup 1.0000x reference)
"""GCN 2-layer forward on 8 Trainium2 NeuronCores (Bass/Tile SPMD).

Math (reference):
    d = rowsum(A) ** -0.5
    A_n = d[:,None] * A * d[None,:]
    out = log_softmax(A_n @ relu(A_n @ X @ W1.T) @ W2.T, axis=1)

Restructured per core i (rows R_i = rows [i*1024, (i+1)*1024)):
    Y1s  = bf16(d[:,None] * (X @ W1.T))                 # fold W1 before the big matmul
    X1   = bf16(relu(d_r[:,None] * (A[R_i,:] @ Y1s)))   # layer-1 rows
    Z2s  = d_r[:,None] * (X1 @ W2.T)  -> AllGather -> Z2s_full
    out_i = log_softmax(d_r[:,None] * (A[R_i,:] @ bf16(Z2s_full)))

Sharding: 1D row partition of A (each core gets its rows, passed transposed so
the contraction index lands on SBUF partitions); X/W replicated; d and the
tiny layer-2 input are AllGathered on device (4KB / 64KB per rank).
"""

import sys

for _p in ("/opt/trn_rl_repo",):
    if _p not in sys.path:
        sys.path.insert(0, _p)

import numpy as np
import ml_dtypes

import concourse.bass as bass
import concourse.mybir as mybir
import concourse.tile as tile
from concourse import bacc
from concourse.bass_utils import run_bass_kernel_spmd
from concourse.masks import make_identity

N, F_IN, HID, CLS = 8192, 512, 256, 16
NCORES = 8
R = N // NCORES          # 1024 rows per core
P = 128
JT = N // P              # 64 contraction chunks
MT = R // P              # 8 local row tiles
FC = F_IN // P           # 4
CC = HID // P            # 2
XSLAB = 512              # streamed X^T slab width (j columns)
NSLAB = N // XSLAB
HALF = 512               # matmul free-dim cap for the d row-sum pass

BF = mybir.dt.bfloat16
F32 = mybir.dt.float32
AF = mybir.ActivationFunctionType
ALU = mybir.AluOpType

TRACE = False
LAST_RESULTS = None


def _emit(tc, at, xt, w1t, w2t, out):
    nc = tc.nc
    rg = [list(range(NCORES))]

    with (
        tc.tile_pool(name="res", bufs=1) as res,
        tc.tile_pool(name="const", bufs=1) as constp,
        tc.tile_pool(name="xstream", bufs=2) as xs,
        tc.tile_pool(name="work", bufs=2) as work,
        tc.tile_pool(name="psum", bufs=4, space="PSUM") as psum,
        tc.tile_pool(name="psd", bufs=2, space="PSUM") as psd,
        tc.tile_pool(name="dram", bufs=1, space="DRAM") as dram,
    ):
        # ---- constants / weights
        w1t_sb = constp.tile([P, FC, HID], BF)
        nc.sync.dma_start(w1t_sb, w1t.rearrange("(fc p) c -> p fc c", p=P))
        w2t_sb = constp.tile([P, CC, CLS], BF)
        nc.sync.dma_start(w2t_sb, w2t.rearrange("(cc p) c -> p cc c", p=P))
        ones_sb = constp.tile([P, 1], BF)
        nc.vector.memset(ones_sb, 1.0)
        ident = constp.tile([P, P], BF)
        make_identity(nc, ident)

        # ---- resident A^T shard (bf16): at [N, R] -> [P, JT, R]
        at_sb = res.tile([P, JT, R], BF)
        atr = at.rearrange("(jt p) m -> p jt m", p=P)
        for s in range(8):
            nc.sync.dma_start(at_sb[:, s * 8:(s + 1) * 8, :], atr[:, s * 8:(s + 1) * 8, :])

        # ---- d_local = 1/sqrt(rowsum(A_bf16)) via ones-matmul (partition reduce)
        d_row = constp.tile([1, R], F32)
        for h in range(R // HALF):
            dps = psd.tile([1, HALF], F32, tag="dps")
            for jt in range(JT):
                nc.tensor.matmul(
                    dps, lhsT=ones_sb, rhs=at_sb[:, jt, h * HALF:(h + 1) * HALF],
                    start=(jt == 0), stop=(jt == JT - 1),
                )
            dsq = work.tile([1, HALF], F32, tag="dsq")
            nc.scalar.activation(dsq, dps, AF.Sqrt)
            nc.vector.reciprocal(d_row[:, h * HALF:(h + 1) * HALF], dsq)

        # ---- AllGather d (4KB per rank)
        d_loc_dram = dram.tile([1, R], F32)
        nc.sync.dma_start(d_loc_dram, d_row)
        d_all_dram = dram.tile([NCORES, R], F32)
        nc.gpsimd.collective_compute(
            "AllGather", ALU.bypass, replica_groups=rg,
            ins=[d_loc_dram.opt()], outs=[d_all_dram.opt()],
        )
        dsb = constp.tile([P, JT], F32)      # d[j], j = jt*128 + p
        dloc = constp.tile([P, MT], F32)     # d[local row], row = mt*128 + p
        with nc.allow_non_contiguous_dma(reason="tiny 32KB d gather"):
            nc.sync.dma_start(dsb, d_all_dram.rearrange("g (jtl p) -> p (g jtl)", p=P))
            nc.sync.dma_start(dloc, d_loc_dram.rearrange("o (mt p) -> p (o mt)", p=P))

        # ---- Y1s = bf16(d[:,None] * (X @ W1.T)), j-tile at a time
        y1s = res.tile([P, JT, HID], BF)
        xtr = xt.rearrange("(fc p) j -> p fc j", p=P)
        for s in range(NSLAB):
            xslab = xs.tile([P, FC, XSLAB], BF, tag="xslab")
            nc.sync.dma_start(xslab, xtr[:, :, s * XSLAB:(s + 1) * XSLAB])
            for k in range(XSLAB // P):
                jt = s * (XSLAB // P) + k
                py = psum.tile([P, HID], F32, tag="mm")
                for fc in range(FC):
                    nc.tensor.matmul(
                        py, lhsT=xslab[:, fc, k * P:(k + 1) * P], rhs=w1t_sb[:, fc, :],
                        start=(fc == 0), stop=(fc == FC - 1),
                    )
                nc.vector.tensor_scalar_mul(y1s[:, jt, :], py, dsb[:, jt:jt + 1])

        # ---- layer 1: X1 = bf16(relu(d_r * (A_i @ Y1s))), then transpose tiles
        x1t = constp.tile([P, CC, R], BF)
        for mt in range(MT):
            ph = psum.tile([P, HID], F32, tag="mm")
            for jt in range(JT):
                nc.tensor.matmul(
                    ph, lhsT=at_sb[:, jt, mt * P:(mt + 1) * P], rhs=y1s[:, jt, :],
                    start=(jt == 0), stop=(jt == JT - 1),
                )
            x1 = work.tile([P, HID], BF, tag="x1")
            nc.scalar.activation(x1, ph, AF.Relu, scale=dloc[:, mt:mt + 1])
            for cc in range(CC):
                pt = psum.tile([P, P], BF, tag="mm")
                nc.tensor.transpose(pt, x1[:, cc * P:(cc + 1) * P], ident)
                nc.vector.tensor_copy(x1t[:, cc, mt * P:(mt + 1) * P], pt)

        # ---- Z2s local rows, AllGather (64KB per rank)
        z2_loc = dram.tile([R, CLS], F32)
        for mt in range(MT):
            pz = psum.tile([P, CLS], F32, tag="mm")
            for cc in range(CC):
                nc.tensor.matmul(
                    pz, lhsT=x1t[:, cc, mt * P:(mt + 1) * P], rhs=w2t_sb[:, cc, :],
                    start=(cc == 0), stop=(cc == CC - 1),
                )
            z2s = work.tile([P, CLS], F32, tag="z2s")
            nc.vector.tensor_scalar_mul(z2s, pz, dloc[:, mt:mt + 1])
            nc.sync.dma_start(z2_loc[mt * P:(mt + 1) * P, :], z2s)
        z2_all = dram.tile([N, CLS], F32)
        nc.gpsimd.collective_compute(
            "AllGather", ALU.bypass, replica_groups=rg,
            ins=[z2_loc.opt()], outs=[z2_all.opt()],
        )
        z2f = constp.tile([P, JT, CLS], F32)
        with nc.allow_non_contiguous_dma(reason="512KB gather, 64B runs"):
            nc.sync.dma_start(z2f, z2_all.rearrange("(jt p) c -> p jt c", p=P))
        z2bf = constp.tile([P, JT, CLS], BF)
        nc.vector.tensor_copy(z2bf, z2f)

        # ---- layer 2 + log_softmax
        for mt in range(MT):
            p2 = psum.tile([P, CLS], F32, tag="mm")
            for jt in range(JT):
                nc.tensor.matmul(
                    p2, lhsT=at_sb[:, jt, mt * P:(mt + 1) * P], rhs=z2bf[:, jt, :],
                    start=(jt == 0), stop=(jt == JT - 1),
                )
            h2 = work.tile([P, CLS], F32, tag="h2")
            nc.vector.tensor_scalar_mul(h2, p2, dloc[:, mt:mt + 1])
            nmx = work.tile([P, 1], F32, tag="nmx")
            nc.vector.reduce_max(nmx, h2, axis=mybir.AxisListType.X, negate=True)
            ex = work.tile([P, CLS], F32, tag="ex")
            ssum = work.tile([P, 1], F32, tag="ssum")
            nc.scalar.activation(ex, h2, AF.Exp, bias=nmx, accum_out=ssum)
            lse = work.tile([P, 1], F32, tag="lse")
            nc.scalar.activation(lse, ssum, AF.Ln)
            o = work.tile([P, CLS], F32, tag="o")
            nc.vector.tensor_scalar(
                o, h2, scalar1=nmx, scalar2=lse, op0=ALU.add, op1=ALU.subtract,
            )
            nc.sync.dma_start(out[mt * P:(mt + 1) * P, :], o)


_NC_CACHE = {}


def _get_nc():
    if "nc" in _NC_CACHE:
        return _NC_CACHE["nc"]
    nc = bacc.Bacc(
        "TRN2",
        target_bir_lowering=False,
        debug=False,
        enable_asserts=True,
        num_devices=NCORES,
    )
    at = nc.dram_tensor("at", [N, R], BF, kind="ExternalInput").ap()
    xt = nc.dram_tensor("xt", [F_IN, N], BF, kind="ExternalInput").ap()
    w1t = nc.dram_tensor("w1t", [F_IN, HID], BF, kind="ExternalInput").ap()
    w2t = nc.dram_tensor("w2t", [HID, CLS], BF, kind="ExternalInput").ap()
    out = nc.dram_tensor("out", [R, CLS], F32, kind="ExternalOutput").ap()
    with tile.TileContext(nc) as tc:
        _emit(tc, at, xt, w1t, w2t, out)
    nc.compile()
    _NC_CACHE["nc"] = nc
    return nc


def make_in_maps(featureMatrix, adjacencyMatrix, W1, W2):
    bf = ml_dtypes.bfloat16
    xtb = np.asarray(featureMatrix).T.astype(bf)
    w1tb = np.asarray(W1).T.astype(bf)
    w2tb = np.asarray(W2).T.astype(bf)
    A = np.asarray(adjacencyMatrix)
    in_maps = []
    for i in range(NCORES):
        atb = A[i * R:(i + 1) * R, :].T.astype(bf)
        in_maps.append({"at": atb, "xt": xtb, "w1t": w1tb, "w2t": w2tb})
    return in_maps


def kernel(featureMatrix, adjacencyMatrix, W1, W2):
    global LAST_RESULTS
    nc = _get_nc()
    in_maps = make_in_maps(featureMatrix, adjacencyMatrix, W1, W2)
    res = run_bass_kernel_spmd(nc, in_maps, core_ids=list(range(NCORES)), trace=TRACE)
    LAST_RESULTS = res
    return np.concatenate([res.results[i]["out"] for i in range(NCORES)], axis=0)


# revision 3
# speedup vs baseline: 1.3072x; 1.3072x over previous
"""GCN 2-layer forward on 8 Trainium2 NeuronCores (Bass/Tile SPMD).

Math (reference):
    d = rowsum(A) ** -0.5
    A_n = d[:,None] * A * d[None,:]
    out = log_softmax(A_n @ relu(A_n @ X @ W1.T) @ W2.T, axis=1)

Restructured per core i (rows R_i = rows [i*1024, (i+1)*1024)):
    Y1s  = bf16(d[:,None] * (X @ W1.T))                 # fold W1 before the big matmul
    X1   = bf16(relu(d_r[:,None] * (A[R_i,:] @ Y1s)))   # layer-1 rows
    Z2s  = d_r[:,None] * (X1 @ W2.T)  -> AllGather -> Z2s_full
    out_i = log_softmax(d_r[:,None] * (A[R_i,:] @ bf16(Z2s_full)))

Sharding: 1D row partition of A (each core gets its rows, passed transposed so
the contraction index lands on SBUF partitions); X/W replicated; d and the
tiny layer-2 input are AllGathered on device (4KB / 64KB per rank).
"""

import sys

for _p in ("/opt/trn_rl_repo",):
    if _p not in sys.path:
        sys.path.insert(0, _p)

import numpy as np
import ml_dtypes

import concourse.bass as bass
import concourse.mybir as mybir
import concourse.tile as tile
from concourse import bacc
from concourse.bass_utils import run_bass_kernel_spmd
from concourse.masks import make_identity

N, F_IN, HID, CLS = 8192, 512, 256, 16
NCORES = 8
R = N // NCORES          # 1024 rows per core
P = 128
JT = N // P              # 64 contraction chunks
MT = R // P              # 8 local row tiles
FC = F_IN // P           # 4
CC = HID // P            # 2
XSLAB = 512              # streamed X^T slab width (j columns)
NSLAB = N // XSLAB
HALF = 512               # matmul free-dim cap for the d row-sum pass

BF = mybir.dt.bfloat16
F32 = mybir.dt.float32
AF = mybir.ActivationFunctionType
ALU = mybir.AluOpType

TRACE = False
LAST_RESULTS = None


def _emit(tc, at, xt, w1t, w2t, out):
    nc = tc.nc
    rg = [list(range(NCORES))]

    with (
        tc.tile_pool(name="res", bufs=1) as res,
        tc.tile_pool(name="const", bufs=1) as constp,
        tc.tile_pool(name="xstream", bufs=2) as xs,
        tc.tile_pool(name="work", bufs=2) as work,
        tc.tile_pool(name="psum", bufs=4, space="PSUM") as psum,
        tc.tile_pool(name="psd", bufs=2, space="PSUM") as psd,
        tc.tile_pool(name="dram", bufs=1, space="DRAM") as dram,
    ):
        # ---- constants / weights
        w1t_sb = constp.tile([P, FC, HID], BF)
        nc.sync.dma_start(w1t_sb, w1t.rearrange("(fc p) c -> p fc c", p=P))
        w2t_sb = constp.tile([P, CC, CLS], BF)
        nc.sync.dma_start(w2t_sb, w2t.rearrange("(cc p) c -> p cc c", p=P))
        ones_sb = constp.tile([P, 1], BF)
        nc.vector.memset(ones_sb, 1.0)
        ident = constp.tile([P, P], BF)
        make_identity(nc, ident)

        # ---- resident A^T shard (bf16): at [N, R] -> [P, JT, R]
        at_sb = res.tile([P, JT, R], BF)
        atr = at.rearrange("(jt p) m -> p jt m", p=P)
        for s in range(8):
            nc.sync.dma_start(at_sb[:, s * 8:(s + 1) * 8, :], atr[:, s * 8:(s + 1) * 8, :])

        # ---- d_local = 1/sqrt(rowsum(A_bf16)) via ones-matmul (partition reduce)
        d_row = constp.tile([1, R], F32)
        for h in range(R // HALF):
            dps = psd.tile([1, HALF], F32, tag="dps")
            for jt in range(JT):
                nc.tensor.matmul(
                    dps, lhsT=ones_sb, rhs=at_sb[:, jt, h * HALF:(h + 1) * HALF],
                    start=(jt == 0), stop=(jt == JT - 1),
                )
            dsq = work.tile([1, HALF], F32, tag="dsq")
            nc.scalar.activation(dsq, dps, AF.Sqrt)
            nc.vector.reciprocal(d_row[:, h * HALF:(h + 1) * HALF], dsq)

        # ---- AllGather d (4KB per rank)
        d_loc_dram = dram.tile([1, R], F32)
        nc.sync.dma_start(d_loc_dram, d_row)
        d_all_dram = dram.tile([NCORES, R], F32)
        nc.gpsimd.collective_compute(
            "AllGather", ALU.bypass, replica_groups=rg,
            ins=[d_loc_dram.opt()], outs=[d_all_dram.opt()],
        )
        dsb = constp.tile([P, JT], F32)      # d[j], j = jt*128 + p
        dloc = constp.tile([P, MT], F32)     # d[local row], row = mt*128 + p
        with nc.allow_non_contiguous_dma(reason="tiny 32KB d gather"):
            nc.sync.dma_start(dsb, d_all_dram.rearrange("g (jtl p) -> p (g jtl)", p=P))
            nc.sync.dma_start(dloc, d_loc_dram.rearrange("o (mt p) -> p (o mt)", p=P))

        # ---- Y1s = bf16(d[:,None] * (X @ W1.T)), j-tile at a time
        y1s = res.tile([P, JT, HID], BF)
        xtr = xt.rearrange("(fc p) j -> p fc j", p=P)
        for s in range(NSLAB):
            xslab = xs.tile([P, FC, XSLAB], BF, tag="xslab")
            nc.sync.dma_start(xslab, xtr[:, :, s * XSLAB:(s + 1) * XSLAB])
            for k in range(XSLAB // P):
                jt = s * (XSLAB // P) + k
                py = psum.tile([P, HID], F32, tag="mm")
                for fc in range(FC):
                    nc.tensor.matmul(
                        py, lhsT=xslab[:, fc, k * P:(k + 1) * P], rhs=w1t_sb[:, fc, :],
                        start=(fc == 0), stop=(fc == FC - 1),
                    )
                nc.vector.tensor_scalar_mul(y1s[:, jt, :], py, dsb[:, jt:jt + 1])

        # ---- layer 1: X1 = bf16(relu(d_r * (A_i @ Y1s))), then transpose tiles
        x1t = constp.tile([P, CC, R], BF)
        for mt in range(MT):
            ph = psum.tile([P, HID], F32, tag="mm")
            for jt in range(JT):
                nc.tensor.matmul(
                    ph, lhsT=at_sb[:, jt, mt * P:(mt + 1) * P], rhs=y1s[:, jt, :],
                    start=(jt == 0), stop=(jt == JT - 1),
                )
            x1 = work.tile([P, HID], BF, tag="x1")
            nc.scalar.activation(x1, ph, AF.Relu, scale=dloc[:, mt:mt + 1])
            for cc in range(CC):
                pt = psum.tile([P, P], BF, tag="mm")
                nc.tensor.transpose(pt, x1[:, cc * P:(cc + 1) * P], ident)
                nc.vector.tensor_copy(x1t[:, cc, mt * P:(mt + 1) * P], pt)

        # ---- Z2s local rows, AllGather (64KB per rank)
        z2_loc = dram.tile([R, CLS], F32)
        for mt in range(MT):
            pz = psum.tile([P, CLS], F32, tag="mm")
            for cc in range(CC):
                nc.tensor.matmul(
                    pz, lhsT=x1t[:, cc, mt * P:(mt + 1) * P], rhs=w2t_sb[:, cc, :],
                    start=(cc == 0), stop=(cc == CC - 1),
                )
            z2s = work.tile([P, CLS], F32, tag="z2s")
            nc.vector.tensor_scalar_mul(z2s, pz, dloc[:, mt:mt + 1])
            nc.sync.dma_start(z2_loc[mt * P:(mt + 1) * P, :], z2s)
        z2_all = dram.tile([N, CLS], F32)
        nc.gpsimd.collective_compute(
            "AllGather", ALU.bypass, replica_groups=rg,
            ins=[z2_loc.opt()], outs=[z2_all.opt()],
        )
        z2f = constp.tile([P, JT, CLS], F32)
        with nc.allow_non_contiguous_dma(reason="512KB gather, 64B runs"):
            nc.sync.dma_start(z2f, z2_all.rearrange("(jt p) c -> p jt c", p=P))
        z2bf = constp.tile([P, JT, CLS], BF)
        nc.vector.tensor_copy(z2bf, z2f)

        # ---- layer 2 + log_softmax
        for mt in range(MT):
            p2 = psum.tile([P, CLS], F32, tag="mm")
            for jt in range(JT):
                nc.tensor.matmul(
                    p2, lhsT=at_sb[:, jt, mt * P:(mt + 1) * P], rhs=z2bf[:, jt, :],
                    start=(jt == 0), stop=(jt == JT - 1),
                )
            h2 = work.tile([P, CLS], F32, tag="h2")
            nc.vector.tensor_scalar_mul(h2, p2, dloc[:, mt:mt + 1])
            nmx = work.tile([P, 1], F32, tag="nmx")
            nc.vector.reduce_max(nmx, h2, axis=mybir.AxisListType.X, negate=True)
            ex = work.tile([P, CLS], F32, tag="ex")
            ssum = work.tile([P, 1], F32, tag="ssum")
            nc.scalar.activation(ex, h2, AF.Exp, bias=nmx, accum_out=ssum)
            lse = work.tile([P, 1], F32, tag="lse")
            nc.scalar.activation(lse, ssum, AF.Ln)
            o = work.tile([P, CLS], F32, tag="o")
            nc.vector.tensor_scalar(
                o, h2, scalar1=nmx, scalar2=lse, op0=ALU.add, op1=ALU.subtract,
            )
            nc.sync.dma_start(out[mt * P:(mt + 1) * P, :], o)


_NC_CACHE = {}


def _get_nc(iters=1):
    key = ("nc", iters)
    if key in _NC_CACHE:
        return _NC_CACHE[key]
    nc = bacc.Bacc(
        "TRN2",
        target_bir_lowering=False,
        debug=False,
        enable_asserts=True,
        num_devices=NCORES,
    )
    at = nc.dram_tensor("at", [N, R], BF, kind="ExternalInput").ap()
    xt = nc.dram_tensor("xt", [F_IN, N], BF, kind="ExternalInput").ap()
    w1t = nc.dram_tensor("w1t", [F_IN, HID], BF, kind="ExternalInput").ap()
    w2t = nc.dram_tensor("w2t", [HID, CLS], BF, kind="ExternalInput").ap()
    out = nc.dram_tensor("out", [R, CLS], F32, kind="ExternalOutput").ap()
    with tile.TileContext(nc) as tc:
        for _ in range(iters):
            _emit(tc, at, xt, w1t, w2t, out)
    nc.compile()
    _NC_CACHE[key] = nc
    return nc


def make_in_maps(featureMatrix, adjacencyMatrix, W1, W2):
    bf = ml_dtypes.bfloat16
    xtb = np.asarray(featureMatrix).T.astype(bf)
    w1tb = np.asarray(W1).T.astype(bf)
    w2tb = np.asarray(W2).T.astype(bf)
    A = np.asarray(adjacencyMatrix)
    in_maps = []
    for i in range(NCORES):
        atb = A[i * R:(i + 1) * R, :].T.astype(bf)
        in_maps.append({"at": atb, "xt": xtb, "w1t": w1tb, "w2t": w2tb})
    return in_maps


def kernel(featureMatrix, adjacencyMatrix, W1, W2):
    global LAST_RESULTS
    nc = _get_nc()
    in_maps = make_in_maps(featureMatrix, adjacencyMatrix, W1, W2)
    res = run_bass_kernel_spmd(nc, in_maps, core_ids=list(range(NCORES)), trace=TRACE)
    LAST_RESULTS = res
    return np.concatenate([res.results[i]["out"] for i in range(NCORES)], axis=0)


# revision 10
# speedup vs baseline: 6.2931x; 4.8143x over previous
"""GCN 2-layer forward on 8 Trainium2 NeuronCores (Bass/Tile SPMD).

Math (reference):
    d = rowsum(A) ** -0.5
    A_n = d[:,None] * A * d[None,:]
    out = log_softmax(A_n @ relu(A_n @ X @ W1.T) @ W2.T, axis=1)

Restructured per core i (rows R_i = rows [i*1024, (i+1)*1024)):
    d_loc = rowsum(A[R_i,:]) ** -0.5                      # local rows only
    Y1s_i = bf16(d_loc[:,None] * (X[R_i,:] @ W1.T))       # own rows, local d
    Y1s   = AllGather(Y1s_i)                              # [N, HID] bf16, 0.5MB/rank
    X1    = bf16(relu(d_loc[:,None] * (A[R_i,:] @ Y1s)))  # layer-1 rows
    Z2s_i = d_loc[:,None] * (X1 @ W2.T) -> AllGather -> Z2s  # 64KB/rank
    out_i = log_softmax(d_loc[:,None] * (A[R_i,:] @ bf16(Z2s)))

No global-d collective is needed: everywhere d appears it scales rows owned
by the core that computes them — the column scaling of A_n rides inside the
gathered Y1s/Z2s.

Sharding: 1D row partition of A; each core receives its row block transposed
(contraction index on SBUF partitions) plus its own X rows transposed;
weights replicated. A^T stays resident in SBUF (16MB bf16).
"""

import sys
import time

for _p in ("/opt/trn_rl_repo",):
    if _p not in sys.path:
        sys.path.insert(0, _p)

import numpy as np
import ml_dtypes

import concourse.bass as bass
import concourse.mybir as mybir
import concourse.tile as tile
from concourse import bacc
from concourse.bass_utils import run_bass_kernel_spmd
from concourse.masks import make_identity

N, F_IN, HID, CLS = 8192, 512, 256, 16
NCORES = 8
R = N // NCORES          # 1024 rows per core
P = 128
JT = N // P              # 64 contraction chunks
MT = R // P              # 8 local row tiles
FC = F_IN // P           # 4
CC = HID // P            # 2
ASLAB = 8                # j-chunks per A^T load slab (8 slabs total)
HALF = 512               # matmul free-dim cap for the d row-sum pass

BF = mybir.dt.bfloat16
F32 = mybir.dt.float32
AF = mybir.ActivationFunctionType
ALU = mybir.AluOpType

TRACE = False
LAST_RESULTS = None


def _emit(tc, at, xts, w1t, w2t, out, fake_coll=False, stop_after=None):
    nc = tc.nc
    rg = [list(range(NCORES))]

    def allgather(in_tile, out_tile):
        # fake_coll: replace the collective with 8 local DMA copies so the
        # program runs single-core for TimelineSim cost modeling.
        if fake_coll:
            for g in range(NCORES):
                nc.sync.dma_start(
                    out_tile[g * in_tile.shape[0]:(g + 1) * in_tile.shape[0]], in_tile
                )
        else:
            nc.gpsimd.collective_compute(
                "AllGather", ALU.bypass, replica_groups=rg,
                ins=[in_tile.opt()], outs=[out_tile.opt()],
            )

    with (
        tc.tile_pool(name="res", bufs=1) as res,
        tc.tile_pool(name="const", bufs=1) as constp,
        tc.tile_pool(name="work", bufs=2) as work,
        tc.tile_pool(name="psum", bufs=4, space="PSUM") as psum,
        tc.tile_pool(name="psd", bufs=2, space="PSUM") as psd,
        tc.tile_pool(name="dram", bufs=1, space="DRAM") as dram,
    ):
        # ---- constants / weights / own-X
        w1t_sb = constp.tile([P, FC, HID], BF)
        nc.sync.dma_start(w1t_sb, w1t.rearrange("(fc p) c -> p fc c", p=P))
        w2t_sb = constp.tile([P, CC, CLS], BF)
        nc.sync.dma_start(w2t_sb, w2t.rearrange("(cc p) c -> p cc c", p=P))
        xts_sb = constp.tile([P, FC, R], BF)
        nc.sync.dma_start(xts_sb, xts.rearrange("(fc p) j -> p fc j", p=P))
        ones_sb = constp.tile([P, 1], BF)
        nc.vector.memset(ones_sb, 1.0)
        ident = constp.tile([P, P], BF)
        make_identity(nc, ident)

        # ---- Y1 own rows, UNSCALED: bf16(X_own @ W1.T) -> AllGather early.
        # No dependence on d, so the gather hides under the A^T load.
        y1_loc = dram.tile([R, HID], BF)
        for k in range(MT):
            py = psum.tile([P, HID], F32, tag="mm")
            for fc in range(FC):
                nc.tensor.matmul(
                    py, lhsT=xts_sb[:, fc, k * P:(k + 1) * P], rhs=w1t_sb[:, fc, :],
                    start=(fc == 0), stop=(fc == FC - 1),
                )
            y1k = work.tile([P, HID], BF, tag="y1k")
            nc.vector.tensor_copy(y1k, py)
            nc.sync.dma_start(y1_loc[k * P:(k + 1) * P, :], y1k)
        y1_all = dram.tile([N, HID], BF, addr_space="Shared")
        allgather(y1_loc, y1_all)
        y1s = res.tile([P, JT, HID], BF)
        nc.sync.dma_start(y1s, y1_all.rearrange("(jt p) c -> p jt c", p=P))

        # ---- resident A^T shard (bf16), slab by slab, with the d row-sum
        # matmuls interleaved per slab so the PE stream follows data arrival.
        at_sb = res.tile([P, JT, R], BF)
        atr = at.rearrange("(jt p) m -> p jt m", p=P)
        nslab = JT // ASLAB
        dps = []
        for h in range(R // HALF):
            dps.append(psd.tile([1, HALF], F32, tag=f"dps{h}", name=f"dps{h}"))
        for s in range(nslab):
            nc.sync.dma_start(
                at_sb[:, s * ASLAB:(s + 1) * ASLAB, :],
                atr[:, s * ASLAB:(s + 1) * ASLAB, :],
            )
            for k in range(ASLAB):
                jt = s * ASLAB + k
                for h in range(R // HALF):
                    nc.tensor.matmul(
                        dps[h], lhsT=ones_sb,
                        rhs=at_sb[:, jt, h * HALF:(h + 1) * HALF],
                        start=(jt == 0), stop=(jt == JT - 1),
                    )

        # d_row = 1/sqrt(rowsums), then redistribute to per-partition columns
        d_row = constp.tile([1, R], F32)
        for h in range(R // HALF):
            dsq = work.tile([1, HALF], F32, tag="dsq")
            nc.scalar.activation(dsq, dps[h], AF.Sqrt)
            nc.vector.reciprocal(d_row[:, h * HALF:(h + 1) * HALF], dsq)
        d_loc_dram = dram.tile([1, R], F32)
        nc.sync.dma_start(d_loc_dram, d_row)
        d_all_dram = dram.tile([NCORES, R], F32, addr_space="Shared")
        allgather(d_loc_dram, d_all_dram)
        dloc = constp.tile([P, MT], F32)     # d[local row], row = mt*128 + p
        dsb = constp.tile([P, JT], F32)      # d[j] for all j, j = jt*128 + p
        with nc.allow_non_contiguous_dma(reason="tiny d redistribute"):
            nc.sync.dma_start(dloc, d_loc_dram.rearrange("o (mt p) -> p (o mt)", p=P))
            nc.sync.dma_start(dsb, d_all_dram.rearrange("g (jtl p) -> p (g jtl)", p=P))
        if stop_after == "d":
            return

        # ---- scale y1s rows in place by the gathered d (pipelines ahead of L1)
        for jt in range(JT):
            nc.vector.tensor_scalar_mul(y1s[:, jt, :], y1s[:, jt, :], dsb[:, jt:jt + 1])
        if stop_after == "y1":
            return

        # ---- layer 1: X1 = bf16(relu(d_loc * (A_i @ Y1s))), then transpose
        x1t = constp.tile([P, CC, R], BF)
        for mt in range(MT):
            ph = psum.tile([P, HID], F32, tag="mm")
            for jt in range(JT):
                nc.tensor.matmul(
                    ph, lhsT=at_sb[:, jt, mt * P:(mt + 1) * P], rhs=y1s[:, jt, :],
                    start=(jt == 0), stop=(jt == JT - 1),
                )
            x1 = work.tile([P, HID], BF, tag="x1")
            nc.scalar.activation(x1, ph, AF.Relu, scale=dloc[:, mt:mt + 1])
            for cc in range(CC):
                pt = psum.tile([P, P], BF, tag="mm")
                nc.tensor.transpose(pt, x1[:, cc * P:(cc + 1) * P], ident)
                nc.vector.tensor_copy(x1t[:, cc, mt * P:(mt + 1) * P], pt)
        if stop_after == "l1":
            return

        # ---- Z2s local rows, AllGather (64KB per rank)
        z2_loc = dram.tile([R, CLS], BF)
        for mt in range(MT):
            pz = psum.tile([P, CLS], F32, tag="mm")
            for cc in range(CC):
                nc.tensor.matmul(
                    pz, lhsT=x1t[:, cc, mt * P:(mt + 1) * P], rhs=w2t_sb[:, cc, :],
                    start=(cc == 0), stop=(cc == CC - 1),
                )
            z2s = work.tile([P, CLS], BF, tag="z2s")
            nc.vector.tensor_scalar_mul(z2s, pz, dloc[:, mt:mt + 1])
            nc.sync.dma_start(z2_loc[mt * P:(mt + 1) * P, :], z2s)
        z2_all = dram.tile([N, CLS], BF, addr_space="Shared")
        allgather(z2_loc, z2_all)
        z2bf = constp.tile([P, JT, CLS], BF)
        with nc.allow_non_contiguous_dma(reason="256KB gather, 32B runs"):
            nc.sync.dma_start(z2bf, z2_all.rearrange("(jt p) c -> p jt c", p=P))
        if stop_after == "z2":
            return

        # ---- layer 2 + log_softmax
        for mt in range(MT):
            p2 = psum.tile([P, CLS], F32, tag="mm")
            for jt in range(JT):
                nc.tensor.matmul(
                    p2, lhsT=at_sb[:, jt, mt * P:(mt + 1) * P], rhs=z2bf[:, jt, :],
                    start=(jt == 0), stop=(jt == JT - 1),
                )
            h2 = work.tile([P, CLS], F32, tag="h2")
            nc.vector.tensor_scalar_mul(h2, p2, dloc[:, mt:mt + 1])
            nmx = work.tile([P, 1], F32, tag="nmx")
            nc.vector.reduce_max(nmx, h2, axis=mybir.AxisListType.X, negate=True)
            ex = work.tile([P, CLS], F32, tag="ex")
            ssum = work.tile([P, 1], F32, tag="ssum")
            nc.scalar.activation(ex, h2, AF.Exp, bias=nmx, accum_out=ssum)
            lse = work.tile([P, 1], F32, tag="lse")
            nc.scalar.activation(lse, ssum, AF.Ln)
            o = work.tile([P, CLS], F32, tag="o")
            nc.vector.tensor_scalar(
                o, h2, scalar1=nmx, scalar2=lse, op0=ALU.add, op1=ALU.subtract,
            )
            nc.sync.dma_start(out[mt * P:(mt + 1) * P, :], o)


_NC_CACHE = {}


def _get_nc(iters=1, stop_after=None):
    key = ("nc", iters, stop_after)
    if key in _NC_CACHE:
        return _NC_CACHE[key]
    nc = bacc.Bacc(
        "TRN2",
        target_bir_lowering=False,
        debug=False,
        enable_asserts=True,
        num_devices=NCORES,
    )
    at = nc.dram_tensor("at", [N, R], BF, kind="ExternalInput").ap()
    xts = nc.dram_tensor("xts", [F_IN, R], BF, kind="ExternalInput").ap()
    w1t = nc.dram_tensor("w1t", [F_IN, HID], BF, kind="ExternalInput").ap()
    w2t = nc.dram_tensor("w2t", [HID, CLS], BF, kind="ExternalInput").ap()
    out = nc.dram_tensor("out", [R, CLS], F32, kind="ExternalOutput").ap()
    with tile.TileContext(nc) as tc:
        for _ in range(iters):
            _emit(tc, at, xts, w1t, w2t, out, stop_after=stop_after)
    nc.compile()
    _NC_CACHE[key] = nc
    return nc


def make_in_maps(featureMatrix, adjacencyMatrix, W1, W2):
    bf = ml_dtypes.bfloat16
    w1tb = np.asarray(W1).T.astype(bf)
    w2tb = np.asarray(W2).T.astype(bf)
    A = np.asarray(adjacencyMatrix)
    X = np.asarray(featureMatrix)
    in_maps = []
    for i in range(NCORES):
        atb = A[i * R:(i + 1) * R, :].T.astype(bf)
        xtsb = X[i * R:(i + 1) * R, :].T.astype(bf)
        in_maps.append({"at": atb, "xts": xtsb, "w1t": w1tb, "w2t": w2tb})
    return in_maps


def kernel(featureMatrix, adjacencyMatrix, W1, W2):
    global LAST_RESULTS
    nc = _get_nc()
    in_maps = make_in_maps(featureMatrix, adjacencyMatrix, W1, W2)
    try:
        res = run_bass_kernel_spmd(
            nc, in_maps, core_ids=list(range(NCORES)), trace=TRACE
        )
    except Exception:
        # The axon terminal occasionally reports the device unrecoverable for
        # ~1 min right after a previous session detached. Reconnect and retry
        # once before giving up.
        time.sleep(90)
        try:
            import jax

            jax.clear_caches()
            jax.extend.backend.clear_backends()
        except Exception:
            pass
        res = run_bass_kernel_spmd(
            nc, in_maps, core_ids=list(range(NCORES)), trace=TRACE
        )
    LAST_RESULTS = res
    return np.concatenate([res.results[i]["out"] for i in range(NCORES)], axis=0)
